# revision 29
# baseline (speedup 1.0000x reference)
"""Multi-head attention (B=2, S=2048, D=1024, H=16) on 8 Trainium2 cores.

Sharding: core = 4*b + g  (b = batch 0..1, g = head-group 0..3, 4 heads each).
Host sums the 4 per-group output partials per batch and adds b_o.

Schedule: software-pipelined.  The softmax exp stream on the ACT engine is
the critical resource (~137us busy); everything else (projections, scores,
attn*V, out-projection) is emitted so the PE feeds/consumes around it and
exp starts within a few microseconds of kernel start.

  - inputs DMA'd in column blocks so Q(chunk0)/K(group0) finish early
  - scores in bf16, two heads row-tiled concurrently in the PE quadrants
  - exp: ACT, [128,2,512] PSUM -> fp8 SBUF tiles (scale folded via `scale=`)
  - attn*V: fp8 DoubleRow matmuls (contraction 256) with an appended
    ones-column accumulating the softmax denominator into row 64
  - normalize: reciprocal_approx_fast + gpsimd partition broadcast
  - out-projection: per-128-query-tile units used as PE filler, bf16 output
"""

import os
from contextlib import ExitStack

import ml_dtypes
import numpy as np

import concourse.bass as bass
import concourse.tile as tile
from concourse import bacc, mybir

B, S, D = 2, 2048, 1024
H, DH = 16, 64
NCORES = 8
NG = 4                  # head-group shards
DG = D // NG            # 256 dims per head-group (4 heads)
P = 128
QC = 512                # q-chunk width
NQC = S // QC           # 4
NKT = S // P            # 16 k-tiles of 128
NKG = NKT // 2          # 8 k-groups of 256 (two 128-tiles)
CD = D // P             # 8 contraction tiles for the projections
F32 = mybir.dt.float32
BF16 = mybir.dt.bfloat16
FP8 = mybir.dt.float8e4
AF = mybir.ActivationFunctionType
SCALE = 1.0 / float(np.sqrt(D))

_TAGS = {}
USE_FP8_U = bool(int(os.environ.get("MHA_FP8_U", "0")))
PT_DT = FP8 if USE_FP8_U else BF16
PT_BUFS = 20 if USE_FP8_U else 18


def _body(ctx: ExitStack, tc: "tile.TileContext", io: dict):
    nc = tc.nc
    ctx.enter_context(nc.allow_low_precision(reason="bf16/fp8 matmul pipeline"))
    sb = ctx.enter_context(tc.tile_pool(name="sb", bufs=1))
    ps = ctx.enter_context(tc.tile_pool(name="ps", bufs=1, space="PSUM"))

    def MM(tag_, *a, **kw):
        mm = nc.tensor.matmul(*a, **kw)
        try:
            _TAGS[mm.ins.name] = tag_
        except Exception:
            pass
        return mm

    # ---- weight / bias DMAs (pre-rearranged on host: contiguous loads) ----
    wq = sb.tile([P, CD, DG], BF16, tag="wq", bufs=1, name="wq")
    nc.sync.dma_start(wq[:], io["wq"][:])
    bq = sb.tile([P, 2], F32, tag="bq", bufs=1, name="bq")
    nc.sync.dma_start(bq[:], io["bq"][:])
    wk = sb.tile([P, CD, DG], BF16, tag="wk", bufs=1, name="wk")
    nc.sync.dma_start(wk[:], io["wk"][:])
    bk = sb.tile([P, 2], F32, tag="bk", bufs=1, name="bk")
    nc.sync.dma_start(bk[:], io["bk"][:])
    ones8 = sb.tile([P, 8], PT_DT, tag="ones8", bufs=1, name="ones8")
    nc.sync.dma_start(ones8[:], io["ones8"][:])
    # preload the exp table set on ACT while input DMAs stream (saves ~2.7us
    # on the first real activation)
    warm = sb.tile([1, 8], F32, tag="warm", bufs=1, name="warm")
    nc.scalar.activation(warm[:], ones8[0:1, :], AF.Exp, scale=1.0)
    # HAM warm-up: ~3.5us of junk matmuls so the PE clock is at 2.4GHz when
    # the first projection lands (reads the wq tile region, content ignored)
    wps = ps.tile([P, QC], F32, tag="aux", bufs=2, name="warmps")
    for i in range(8):
        MM(
            "warmup",
            wps[:],
            wq[:, 0, 0:P],
            wq[:].rearrange("p a b -> p (a b)")[:, 0:QC],
            start=(i == 0),
            stop=(i == 7),
        )

    # ---- input DMAs: host-relayout [P, blk, c, q] -> one 1MB DMA per block
    # (Sync-engine dma_start issue costs ~0.8us each; few big DMAs keep the
    # issue path off the critical path and the wires at full rate)
    xq = sb.tile([P, NQC, CD, QC], BF16, tag="xq", bufs=1, name="xq")
    xkb = [sb.tile([P, CD, QC], BF16, tag="xk", bufs=2, name=f"xk{b}") for b in range(NQC)]
    xvb = [sb.tile([P, CD, QC], BF16, tag="xv", bufs=2, name=f"xv{b}") for b in range(NQC)]

    def dma_x_block(name, t, blk):
        if name == "xq":
            nc.sync.dma_start(t[:, blk], io[name][:, blk])
        else:
            nc.sync.dma_start(t[blk][:], io[name][:, blk])

    # exp-stream-critical first: xq block0 (Q chunk 0) and xk block0; then
    # xk and xv interleaved (xk feeds the exp stream via scores, xv feeds the
    # U matmuls that recycle pt tiles — both must keep pace with ACT).
    wv = sb.tile([P, CD, DG], BF16, tag="wv", bufs=1, name="wv")
    bvb = sb.tile([P, DG], BF16, tag="bvb", bufs=1, name="bvb")

    dma_x_block("xq", xq, 0)
    dma_x_block("xk", xkb, 0)
    nc.sync.dma_start(wv[:], io["wv"][:])
    nc.sync.dma_start(bvb[:], io["bvb"][:])
    dma_x_block("xv", xvb, 0)
    dma_x_block("xk", xkb, 1)
    dma_x_block("xv", xvb, 1)
    dma_x_block("xq", xq, 1)
    dma_x_block("xk", xkb, 2)
    dma_x_block("xv", xvb, 2)
    dma_x_block("xk", xkb, 3)
    dma_x_block("xv", xvb, 3)

    woT = []
    for pr in range(2):
        t = sb.tile([P, D], BF16, tag="wo", bufs=2, name=f"woT{pr}")
        nc.sync.dma_start(t[:], io["wo"][pr * P : (pr + 1) * P, :])
        woT.append(t)
    dma_x_block("xq", xq, 2)
    dma_x_block("xq", xq, 3)

    # ---- emission helpers --------------------------------------------------
    QT = {}           # (pr, qc) -> [128, 512] bf16
    KT = [None, None]  # pr -> [128, S] bf16
    for pr in range(2):
        KT[pr] = sb.tile([P, S], BF16, tag="kt", bufs=2, name=f"KT{pr}")
    VA = {}           # g -> [128, 2, 4, 65] fp8/bf16 (ki, kk, head, dim+ones)
    PT = {}           # (qc, h, g) -> [128, 2, 512] pt tile
    U = {}            # (qc, h) -> [65, 512] f32 psum
    UN = {}           # (qc, pair) -> [128, 512] bf16
    YSB = {}

    def emit_q_half(qc, pr):
        psg = ps.tile([P, QC], F32, tag="aux", bufs=2, name=f"psq{qc}_{pr}")
        for c in range(CD):
            MM(
                "qproj",
                psg[:],
                wq[:, c, pr * P : (pr + 1) * P],
                xq[:, qc, c, :],
                start=(c == 0),
                stop=(c == CD - 1),
            )
        t = sb.tile([P, QC], BF16, tag="qt", bufs=4, name=f"QT{qc}_{pr}")
        nc.vector.tensor_scalar_add(t[:], psg[:], bq[:, pr : pr + 1])
        QT[pr, qc] = t

    def emit_q_unit(qc):
        for pr in range(2):
            emit_q_half(qc, pr)

    def emit_k_unit(pr, gp):
        # one unit covers a g-pair: k-columns [gp*512, (gp+1)*512)
        psg = ps.tile([P, QC], F32, tag="aux", bufs=2, name=f"psk{pr}_{gp}")
        for c in range(CD):
            MM(
                "kproj",
                psg[:],
                wk[:, c, pr * P : (pr + 1) * P],
                xkb[gp][:, c, :],
                start=(c == 0),
                stop=(c == CD - 1),
            )
        nc.vector.tensor_scalar_add(
            KT[pr][:, gp * QC : (gp + 1) * QC], psg[:], bk[:, pr : pr + 1]
        )

    def emit_v_unit(g):
        # V rows for keys [g*256, (g+1)*256): two 128-row tiles (j=0,1).
        # Separate PSUM tiles per j: two start=True groups may not share a
        # bank (the second start clears the whole bank's has_written bits).
        psv = [
            ps.tile([P, DG], F32, tag="aux", bufs=2, name=f"psv{g}_{j}")
            for j in range(2)
        ]
        for c in range(CD):
            for j in range(2):
                st_i = g * 2 + j
                MM(
                    "vproj",
                    psv[j][:],
                    xvb[st_i // 4][:, c, (st_i % 4) * P : (st_i % 4 + 1) * P],
                    wv[:, c, :],
                    start=(c == 0),
                    stop=(c == CD - 1),
                )
        # fp8 DoubleRow: stationary padded to 128 columns (LDWEIGHTS requires
        # col_grp=0xf and a 16-aligned ko step); U rows 65..127 are garbage
        # and never read.  bf16: compact 65-column stationary.
        vw = P if USE_FP8_U else DH + 1
        vt = sb.tile([P, 2, 4, vw], PT_DT, tag="va", bufs=NKG, name=f"VA{g}")
        if USE_FP8_U:
            nc.gpsimd.memset(
                vt[:].rearrange("p a b c -> p (a b) c")[:, :, DH : vw], 0
            )
        for j in range(2):
            nc.vector.tensor_add(
                vt[:, j, :, 0:DH],
                psv[j][:].rearrange("p (h d) -> p h d", h=4),
                bvb[:].rearrange("p (h d) -> p h d", h=4),
            )
        nc.vector.tensor_copy(
            vt[:].rearrange("p a b c -> p (a b) c")[:, :, DH : DH + 1],
            ones8[:, :, None],
        )
        VA[g] = vt

    def emit_scores_exp(qc, pair, g):
        # One ST tile per (g, k-tile) holding BOTH heads of the pair (dim1 =
        # head).  The pair's two matmuls hit different PE row-halves and
        # become ready on the same slot release, so they run concurrently.
        pr = pair
        for kk in range(2):
            st = ps.tile([P, 2, QC], F32, tag="st", bufs=2, name=f"st{qc}_{g}_{kk}")
            k_tile = g * 2 + kk
            for i, h in enumerate((2 * pair, 2 * pair + 1)):
                lo = (h % 2) * 64
                MM(
                    "scores",
                    st[:, i, :],
                    KT[pr][lo : lo + 64, k_tile * P : (k_tile + 1) * P],
                    QT[pr, qc][lo : lo + 64, :],
                    start=True,
                    stop=True,
                    tile_position=(lo, 0),
                )
            pt = sb.tile(
                [P, 2, QC], PT_DT, tag="pt", bufs=PT_BUFS, name=f"pt{qc}_{g}_{kk}"
            )
            nc.scalar.activation(pt[:], st[:], AF.Exp, scale=SCALE)
            PT[qc, g, kk] = pt

    def emit_u(qc, h, g):
        if (qc, h) not in U:
            U[qc, h] = ps.tile([DH + 1, QC], F32, tag="u", bufs=2, name=f"U{qc}_{h}")
        i_h = h % 2
        for kk in range(2):
            MM(
                "u",
                U[qc, h][:],
                VA[g][:, kk, h, 0 : DH + 1],
                PT[qc, g, kk][:, i_h, :],
                start=(g == 0 and kk == 0),
                stop=(g == NKG - 1 and kk == 1),
            )
        if i_h == 1:  # second head of the pair consumed both pt tiles
            del PT[qc, g, 0], PT[qc, g, 1]

    def emit_normalize(qc, pair):
        # UN rows 0..63 = head 2*pair, rows 64..127 = head 2*pair+1 (DMA shift)
        un = sb.tile([P, QC], BF16, tag="un", bufs=6, name=f"UN{qc}_{pair}")
        UN[qc, pair] = un
        for i, h in ((1, 2 * pair + 1), (0, 2 * pair)):
            u = U.pop((qc, h))
            # evacuate U from PSUM right away: the bank is the scarce
            # resource gating the next pair's accumulation
            ucp = sb.tile([65, QC], F32, tag="ucp", bufs=4, name=f"ucp{qc}_{h}")
            nc.vector.tensor_copy(ucp[:], u[0:65, :])
            z0 = sb.tile([1, QC], F32, tag="z0", bufs=2, name=f"z0_{qc}_{h}")
            nc.sync.dma_start(z0[:], ucp[64:65, :])
            rz = sb.tile([1, QC], F32, tag="rz", bufs=2, name=f"rz{qc}_{h}")
            nc.vector.reciprocal_approx_fast(rz[:], z0[:])
            rb = sb.tile([64, QC], F32, tag="rb", bufs=2, name=f"rb{qc}_{h}")
            nc.gpsimd.partition_broadcast(rb[:], rz[:], channels=64)
            if i == 0:
                nc.vector.tensor_mul(un[0:64, :], ucp[0:64, :], rb[:])
            else:
                tmp = sb.tile([64, QC], BF16, tag="untmp", bufs=2, name=f"ut{qc}_{h}")
                nc.vector.tensor_mul(tmp[:], ucp[0:64, :], rb[:])
                nc.sync.dma_start(un[64:128, :], tmp[:])

    def emit_outproj_unit(qcp, qi, ec):
        qt = qcp * 4 + qi
        if ec == 0:
            YSB[qt] = sb.tile([P, D], BF16, tag="y", bufs=4, name=f"Y{qt}")
        ysb = YSB[qt]
        yps = ps.tile([P, QC], F32, tag="aux", bufs=2, name=f"yp{qt}_{ec}")
        for pr in range(2):
            MM(
                "oproj",
                yps[:],
                UN[qcp, pr][:, qi * P : (qi + 1) * P],
                woT[pr][:, ec * QC : (ec + 1) * QC],
                start=(pr == 0),
                stop=(pr == 1),
            )
        nc.vector.tensor_copy(ysb[:, ec * QC : (ec + 1) * QC], yps[:])
        if ec == 1:
            nc.sync.dma_start(io["y"][qt * P : (qt + 1) * P, :], ysb[:])

    # ---- main schedule ----------------------------------------------------
    # Per q-chunk: two head pairs, 8 k-groups each.  Projections (K, V) and
    # the U/out-projection consumers are spread through the exp stream as PE
    # filler; U lags exp by up to a pair-sweep (pt tiles buffer).
    u_backlog = []      # (qc, h, g) ready to emit once VA[g] exists
    op_backlog = []     # (qc, qi, ec) out-projection units
    u_done = {}         # (qc, pair) -> count of emitted U matmuls (of 16)
    norm_queue = []     # (qc, pair) pending normalize, FIFO

    def drain_u(maxn=None):
        n = 0
        while u_backlog and (maxn is None or n < maxn):
            qc_, h_, g_ = u_backlog[0]
            if g_ not in VA:
                break
            u_backlog.pop(0)
            emit_u(qc_, h_, g_)
            key = (qc_, h_ // 2)
            u_done[key] = u_done.get(key, 0) + 1
            n += 1
        # emit any normalize whose pair is fully accumulated
        while norm_queue and u_done.get(norm_queue[0], 0) == 2 * NKG:
            emit_normalize(*norm_queue.pop(0))

    def drain_op(maxn):
        n = 0
        while op_backlog and n < maxn:
            qcp = op_backlog[0][0]
            if (qcp, 0) in norm_queue or (qcp, 1) in norm_queue:
                break  # UN for that chunk not emitted yet
            emit_outproj_unit(*op_backlog.pop(0))
            n += 1

    emit_q_half(0, 0)
    emit_k_unit(0, 0)

    for qc in range(NQC):
        for pair in range(2):
            for g in range(NKG):
                emit_scores_exp(qc, pair, g)
                for h in (2 * pair, 2 * pair + 1):
                    u_backlog.append((qc, h, g))
                # --- PE filler work, paced with the exp stream ---
                if qc == 0:
                    if pair == 0:
                        if g % 2 == 0 and g // 2 + 1 < NKG // 2:
                            emit_k_unit(0, g // 2 + 1)  # K(pr0) ahead of scores
                        if g % 2 == 0:
                            emit_v_unit(g // 2)         # V units: 4 in pair0
                        if g == 1:
                            emit_q_half(0, 1)
                        if g % 2 == 1 and (g - 3) // 2 + 1 <= 3:
                            emit_k_unit(1, max(0, (g - 3) // 2 + 1) if g >= 3 else 0)
                    else:
                        if g % 2 == 0:
                            emit_v_unit(4 + g // 2)
                        elif g == 1:
                            emit_q_half(1, 0)
                        elif g == 3:
                            emit_q_half(1, 1)
                elif qc == 1 and pair == 0 and g == 0:
                    emit_q_half(2, 0)
                elif qc == 1 and pair == 0 and g == 1:
                    emit_q_half(2, 1)
                elif qc == 2 and pair == 0 and g == 0:
                    emit_q_half(3, 0)
                elif qc == 2 and pair == 0 and g == 1:
                    emit_q_half(3, 1)
                drain_u(2 if qc == 0 else 3)
                drain_op(1 if qc >= 1 else 0)
            norm_queue.append((qc, pair))
            drain_u()
        op_backlog.extend((qc, qi, ec) for qi in range(4) for ec in range(2))

    drain_u()
    assert not u_backlog and not norm_queue, (u_backlog, norm_queue)
    drain_op(len(op_backlog))


def build_program():
    nc = bacc.Bacc(
        "TRN2", target_bir_lowering=False, debug=False, num_devices=NCORES
    )
    io = {
        "xq": nc.dram_tensor("xq", [P, NQC, CD, QC], BF16, kind="ExternalInput").ap(),
        "xk": nc.dram_tensor("xk", [P, NQC, CD, QC], BF16, kind="ExternalInput").ap(),
        "xv": nc.dram_tensor("xv", [P, NQC, CD, QC], BF16, kind="ExternalInput").ap(),
        "wq": nc.dram_tensor("wq", [P, CD, DG], BF16, kind="ExternalInput").ap(),
        "wk": nc.dram_tensor("wk", [P, CD, DG], BF16, kind="ExternalInput").ap(),
        "wv": nc.dram_tensor("wv", [P, CD, DG], BF16, kind="ExternalInput").ap(),
        "wo": nc.dram_tensor("wo", [DG, D], BF16, kind="ExternalInput").ap(),
        "bq": nc.dram_tensor("bq", [P, 2], F32, kind="ExternalInput").ap(),
        "bk": nc.dram_tensor("bk", [P, 2], F32, kind="ExternalInput").ap(),
        "bvb": nc.dram_tensor("bvb", [P, DG], BF16, kind="ExternalInput").ap(),
        "ones8": nc.dram_tensor("ones8", [P, 8], PT_DT, kind="ExternalInput").ap(),
        "y": nc.dram_tensor("y", [S, D], BF16, kind="ExternalOutput").ap(),
    }
    with tile.TileContext(nc) as tc:
        with ExitStack() as ctx:
            _body(ctx, tc, io)
    nc.compile()
    try:
        import json
        with open("/tmp/mha_tags.json", "w") as f:
            json.dump(_TAGS, f)
    except Exception:
        pass
    return nc


_CACHE = {}


def _get_program():
    if "nc" not in _CACHE:
        _CACHE["nc"] = build_program()
    return _CACHE["nc"]


def make_in_maps(inputs):
    q = np.asarray(inputs["query"], np.float32)
    k = np.asarray(inputs["key"], np.float32)
    v = np.asarray(inputs["value"], np.float32)
    W_q = np.asarray(inputs["W_q"], np.float32)
    W_k = np.asarray(inputs["W_k"], np.float32)
    W_v = np.asarray(inputs["W_v"], np.float32)
    W_o = np.asarray(inputs["W_o"], np.float32)
    b_q = np.asarray(inputs["b_q"], np.float32)
    b_k = np.asarray(inputs["b_k"], np.float32)
    b_v = np.asarray(inputs["b_v"], np.float32)

    bf = ml_dtypes.bfloat16
    f8 = ml_dtypes.float8_e4m3
    def xarr(x, b):
        # x[b].T [D, S] -> [P, NQC, CD, QC]; element (p, blk, c, q) =
        # xT[c*P + p, blk*QC + q]
        xT = x[b].T.reshape(CD, P, NQC, QC).transpose(1, 2, 0, 3)
        return np.ascontiguousarray(xT).astype(bf)

    xT = [[xarr(x, b) for b in range(B)] for x in (q, k, v)]

    def warr(W, sl):
        # W[sl].T [D, DG] -> [P, CD, DG] with row c*P+p at [p, c, :]
        wt = W[sl, :].T.reshape(CD, P, DG).transpose(1, 0, 2)
        return np.ascontiguousarray(wt).astype(bf)

    def barr(b, sl):
        return np.ascontiguousarray(b[sl].reshape(2, P).T)

    in_maps = []
    for core in range(NCORES):
        b, g = divmod(core, NG)
        sl = slice(g * DG, (g + 1) * DG)
        in_maps.append(
            {
                "xq": xT[0][b],
                "xk": xT[1][b],
                "xv": xT[2][b],
                "wq": warr(W_q, sl),
                "wk": warr(W_k, sl),
                "wv": warr(W_v, sl),
                "wo": np.ascontiguousarray(W_o[:, sl].T).astype(bf),
                "bq": barr(b_q, sl),
                "bk": barr(b_k, sl),
                "bvb": np.tile(b_v[sl][None, :], (P, 1)).astype(bf),
                "ones8": np.ones((P, 8), f8 if USE_FP8_U else bf),
            }
        )
    return in_maps


def kernel(**inputs):
    from concourse.bass_utils import run_bass_kernel_spmd

    nc = _get_program()
    in_maps = make_in_maps(inputs)
    trace = bool(int(os.environ.get("MHA_TRACE", "0")))
    res = run_bass_kernel_spmd(nc, in_maps, list(range(NCORES)), trace=trace)
    _CACHE["last_results"] = res

    b_o = np.asarray(inputs["b_o"], np.float32)
    out = np.zeros((B, S, D), np.float32)
    for core in range(NCORES):
        b = core // NG
        out[b] += res.results[core]["y"].astype(np.float32)
    out += b_o[None, None, :]
    return out
